# revision 46
# baseline (speedup 1.0000x reference)
"""MinEntropyConsensusLoss Trainium2 kernel (bf16 DVE-fold pipeline).

loss = 0.5 * mean_b( min_c( -log_softmax(x)[b,c] - log_softmax(y)[b,c] ) )
     = 0.5 * mean_b( lse(x_b) + lse(y_b) - max_c(x[b,c] + y[b,c]) )

Uses max_c(x+y) = ln(max_c(exp(x)*exp(y))) so the exp tiles (needed for
lse anyway) feed the max path and no separate x+y add is ever needed.

Data-parallel over 8 NeuronCores; each core streams 16384 rows as 16
chunks of [128 partitions x 8 rows x 256 cols]; DMA floor ~84us/core at
the ~408 GB/s per-core cap (ambient HBM contention can degrade this to
~300 GB/s -> ~113us; the kernel is then purely DMA-bound). Per chunk:
  ACT  exp(x), exp(y) fp32 -> bf16 batched [128,2048]: 2 x ~2.0us
  DVE  product p = ex*ey (bf16 TT, 2x mode, ~1.2us), then binary FOLD
       chains (TT add/max over strided 3D halves) + one small reduce:
       sums [128,16,256]->[128,16] ~3.0us, maxes [128,8,256]->[128,8]
       ~1.8us. Total ~6.1us/chunk - the engine pacer at full DMA rate.
First and last chunks run as half-chunks (first also quarter-DMAs) to
shorten pipeline fill/drain. Stats land f32 in [128,384]; tail does
Ln(sums), Ln(maxprod), two row-reduces, subtract -> col 0 of a
[128,1024] padded output (host sums col 0 over cores/partitions).

Hardware facts measured on TRN2 silicon (hold for future edits):
  - DVE TENSOR_TENSOR with ALL-bf16 packed operands runs 2x
    (0.59ns/elem); fp32 or mixed dtype is 1x (1.04). TENSOR_REDUCE is
    ALWAYS 1x regardless of dtype/shape - hence fold-then-small-reduce.
  - GPSIMD software ops (TENSOR_TENSOR) STALL concurrent DVE
    instructions to ~zero throughput for their whole duration
    (1:1 loss) - keep GPSIMD idle, never overlap it with DVE.
  - ACT accum_out costs ~280ns/READ_ACCUMULATOR; [128,1]-granular
    only - useless for per-row sums at T rows/partition.
  - DMAs must issue from nc.sync's ring; ACT's HWDGE ring serializes
    DMAs behind its compute instructions (+15us measured here).
  - A small (512B) SBUF->HBM output transfer sits ~12-14us in the
    write queue (aggregation path) and that lag lands INSIDE the
    profiled exec window; padding the out to 4KB/partition fixes it.
  - vector.tensor_tensor_reduce with op1=max/min WEDGES the core.
  - Custom DVE ops (dve_ops.OPS) all run 1x (no perf_en) and accum_out
    is [128,1]-granular: no win over native TT folds.
"""

import sys

sys.path.insert(0, "/opt/trn_rl_repo")

import numpy as np

import concourse.bacc as bacc
import concourse.mybir as mybir
import concourse.tile as tile
from concourse.bass_utils import run_bass_kernel_spmd

B, C = 131072, 256
N_CORES = 8
R = B // N_CORES          # rows per core = 16384
T = 8                     # rows per partition per chunk
CH = T * C                # 2048 free elems per tensor per chunk
NCH = R // (128 * T)      # 16 chunks per core
NG = R // 128             # 128 row-groups (max values) per core
BUFS = 6                  # input tile ring depth
EBUFS = 3                 # combined-exp tile ring depth (2-chunk supers)
PBUFS = 2                 # product tile ring depth

_cache = {}


def _build_nc(repeat=1):
    f32 = mybir.dt.float32
    bf16 = mybir.dt.bfloat16
    A = mybir.AluOpType
    Exp = mybir.ActivationFunctionType.Exp
    Ln = mybir.ActivationFunctionType.Ln
    X = mybir.AxisListType.X
    nc = bacc.Bacc("TRN2", target_bir_lowering=False, debug=False)
    x_d = nc.dram_tensor("x", [R, C], f32, kind="ExternalInput")
    y_d = nc.dram_tensor("y", [R, C], f32, kind="ExternalInput")
    # [128, 1024] instead of [128, 1]: 512-byte output transfers sit
    # ~12-14us in the SBUF->HBM queue (small-transfer aggregation path);
    # a 4KB-per-partition transfer executes promptly. Host reads col 0.
    out_d = nc.dram_tensor("out", [128, 1024], f32, kind="ExternalOutput")

    # chunk c, partition p holds rows c*1024 + p*T + t (t contiguous)
    x_v = x_d.ap().rearrange("(c p t) f -> c p (t f)", c=NCH, p=128, t=T)
    y_v = y_d.ap().rearrange("(c p t) f -> c p (t f)", c=NCH, p=128, t=T)

    with tile.TileContext(nc) as tc:
        with (
            tc.tile_pool(name="xin", bufs=BUFS) as xin_pool,
            tc.tile_pool(name="yin", bufs=BUFS) as yin_pool,
            tc.tile_pool(name="exp", bufs=EBUFS) as e_pool,
            tc.tile_pool(name="prod", bufs=PBUFS) as p_pool,
            tc.tile_pool(name="fold", bufs=1) as f_pool,
            tc.tile_pool(name="stats", bufs=1) as stats_pool,
        ):
            sxy_t = stats_pool.tile([128, 2 * NG], f32, tag="sxy")
            mx_t = stats_pool.tile([128, NG], f32, tag="mx")
            obig_t = stats_pool.tile([128, 1024], f32, tag="obig")
            nc.vector.memset(obig_t[:], 0.0)


            def half(view_3d, f):
                # [128, t, 2f] -> two [128, t, f] halves
                t = view_3d.shape[1]
                v4 = view_3d.rearrange("p t (h f) -> p t h f", h=2, f=f)
                return v4[:, :, 0], v4[:, :, 1]

            cols = {"sxy": 0, "mx": 0}

            def chain(src_view, nt, cap, op, red, stats_t, ckey, tpre, nfold):
                # src_view: [128, nt*256] 2D AP; nfold bf16 TT folds then
                # one reduce into the next nt cols of stats_t (f32).
                # Fold tiles are fixed cap-size in a bufs=1 pool: safe,
                # all DVE in-order.
                v = src_view.rearrange("p (t f) -> p t f", t=nt)
                f = 128
                for i in range(nfold):
                    t_i = f_pool.tile([128, cap * f], bf16, tag=f"{tpre}{i}")
                    a, b = half(v, f)
                    v = t_i[:, : nt * f].rearrange("p (t f) -> p t f", t=nt)
                    nc.vector.tensor_tensor(out=v, in0=a, in1=b, op=op)
                    f //= 2
                c0 = cols[ckey]
                cols[ckey] = c0 + nt
                red(stats_t[:, c0 : c0 + nt], v, axis=X)

            def sum_chain(e_view, nt):
                chain(e_view, nt, 32, A.add, nc.vector.reduce_sum,
                      sxy_t, "sxy", "s", 3)

            def max_chain(p_view, nt):
                chain(p_view, nt, 16, A.max, nc.vector.reduce_max,
                      mx_t, "mx", "m", 2)

            def half_chunk(c, hh, quarters=False):
                # one half-chunk (4 rows/partition): fast pipeline
                # fill (c==0) and short drain (c==NCH-1). Reuses the
                # full-size tile tags (partial use) to save SBUF.
                hq = CH // 2
                xs = x_v[c][:, hh * hq : (hh + 1) * hq]
                ys = y_v[c][:, hh * hq : (hh + 1) * hq]
                x_t = xin_pool.tile([128, CH], f32, tag="x")
                y_t = yin_pool.tile([128, CH], f32, tag="y")
                if quarters:
                    # interleave x/y quarter-DMAs so both exps can
                    # start as early as possible on the very first chunk
                    q = hq // 2
                    nc.sync.dma_start(x_t[:, :q], xs[:, :q])
                    nc.sync.dma_start(y_t[:, :q], ys[:, :q])
                    nc.sync.dma_start(x_t[:, q:hq], xs[:, q:])
                    nc.sync.dma_start(y_t[:, q:hq], ys[:, q:])
                else:
                    nc.sync.dma_start(x_t[:, :hq], xs)
                    nc.sync.dma_start(y_t[:, :hq], ys)

                e_t = e_pool.tile([128, 4 * CH], bf16, tag="e")
                if quarters:
                    q = hq // 2
                    nc.scalar.activation(e_t[:, :q], x_t[:, :q], Exp)
                    nc.scalar.activation(e_t[:, hq : hq + q], y_t[:, :q], Exp)
                    nc.scalar.activation(e_t[:, q:hq], x_t[:, q:hq], Exp)
                    nc.scalar.activation(e_t[:, hq + q : 2 * hq], y_t[:, q:hq], Exp)
                else:
                    nc.scalar.activation(e_t[:, :hq], x_t[:, :hq], Exp)
                    nc.scalar.activation(e_t[:, hq : 2 * hq], y_t[:, :hq], Exp)

                p_t = p_pool.tile([128, 2 * CH], bf16, tag="p")
                nc.vector.tensor_tensor(
                    out=p_t[:, :hq], in0=e_t[:, :hq],
                    in1=e_t[:, hq : 2 * hq], op=A.mult,
                )
                sum_chain(e_t[:, : 2 * hq], T)
                max_chain(p_t[:, :hq], T // 2)

            def one_pass():
                half_chunk(0, 0, quarters=True)
                half_chunk(0, 1)
                # 2-chunk super-chunks: double-size DVE fold instructions
                # halve the small-instruction overhead and merge the
                # final reduces (~0.5us/chunk off the DVE pacer).
                # Product on DVE: GPSIMD's software TENSOR_TENSOR stalls
                # concurrent DVE instructions to ~zero throughput.
                for s in range(1, NCH - 1, 2):
                    xa = xin_pool.tile([128, CH], f32, tag="x")
                    ya = yin_pool.tile([128, CH], f32, tag="y")
                    nc.sync.dma_start(xa[:], x_v[s])
                    nc.sync.dma_start(ya[:], y_v[s])
                    xb = xin_pool.tile([128, CH], f32, tag="x")
                    yb = yin_pool.tile([128, CH], f32, tag="y")
                    nc.sync.dma_start(xb[:], x_v[s + 1])
                    nc.sync.dma_start(yb[:], y_v[s + 1])

                    # layout [exa | exb | eya | eyb] so the product's
                    # operands are the two contiguous 2-chunk halves
                    e_t = e_pool.tile([128, 4 * CH], bf16, tag="e")
                    nc.scalar.activation(e_t[:, :CH], xa[:], Exp)
                    nc.scalar.activation(e_t[:, 2 * CH : 3 * CH], ya[:], Exp)
                    nc.scalar.activation(e_t[:, CH : 2 * CH], xb[:], Exp)
                    nc.scalar.activation(e_t[:, 3 * CH :], yb[:], Exp)

                    p_t = p_pool.tile([128, 2 * CH], bf16, tag="p")
                    nc.vector.tensor_tensor(
                        out=p_t[:], in0=e_t[:, : 2 * CH],
                        in1=e_t[:, 2 * CH :], op=A.mult,
                    )

                    sum_chain(e_t[:], 4 * T)
                    max_chain(p_t[:], 2 * T)

                # last chunk as two half-chunks to shorten the drain
                half_chunk(NCH - 1, 0)
                half_chunk(NCH - 1, 1)

            if repeat > 1:
                with tc.For_i(0, repeat, 1):
                    one_pass()
            else:
                one_pass()

            # --- device tail: stats -> [128, 1] partial sum ---
            ln_t = stats_pool.tile([128, 2 * NG], f32, tag="ln")
            mln_t = stats_pool.tile([128, NG], f32, tag="mln")
            ts_t = stats_pool.tile([128, 1], f32, tag="ts")
            tm_t = stats_pool.tile([128, 1], f32, tag="tm")
            nc.scalar.activation(ln_t[:], sxy_t[:], Ln)
            # mx holds max(ex*ey) = exp(max(x+y)); ln() recovers the max
            nc.scalar.activation(mln_t[:], mx_t[:], Ln)
            nc.vector.reduce_sum(ts_t[:], ln_t[:], axis=X)
            nc.vector.reduce_sum(tm_t[:], mln_t[:], axis=X)
            nc.vector.tensor_tensor(
                out=obig_t[:, :1], in0=ts_t[:], in1=tm_t[:], op=A.subtract
            )
            nc.sync.dma_start(out_d.ap(), obig_t[:])

    nc.compile()
    return nc


def get_nc():
    if "nc" not in _cache:
        _cache["nc"] = _build_nc()
    return _cache["nc"]


def run_cores(x, y, **kw):
    nc = get_nc()
    x = np.ascontiguousarray(np.asarray(x, dtype=np.float32))
    y = np.ascontiguousarray(np.asarray(y, dtype=np.float32))
    in_maps = [
        {"x": x[k * R : (k + 1) * R], "y": y[k * R : (k + 1) * R]}
        for k in range(N_CORES)
    ]
    return run_bass_kernel_spmd(nc, in_maps, list(range(N_CORES)), **kw)


def kernel(x, y):
    res = run_cores(x, y)
    total = 0.0
    for r in res.results:
        total += float(np.sum(r["out"][:, 0].astype(np.float64)))
    return np.float32(0.5 * total / B)


# revision 47
# speedup vs baseline: 1.0425x; 1.0425x over previous
"""MinEntropyConsensusLoss Trainium2 kernel (bf16 DVE-fold pipeline).

loss = 0.5 * mean_b( min_c( -log_softmax(x)[b,c] - log_softmax(y)[b,c] ) )
     = 0.5 * mean_b( lse(x_b) + lse(y_b) - max_c(x[b,c] + y[b,c]) )

Uses max_c(x+y) = ln(max_c(exp(x)*exp(y))) so the exp tiles (needed for
lse anyway) feed the max path and no separate x+y add is ever needed.

Data-parallel over 8 NeuronCores; each core streams 16384 rows as 16
chunks of [128 partitions x 8 rows x 256 cols]; DMA floor ~84us/core at
the ~408 GB/s per-core cap (ambient HBM contention can degrade this to
~300 GB/s -> ~113us; the kernel is then purely DMA-bound). Per chunk:
  ACT  exp(x), exp(y) fp32 -> bf16 batched [128,2048]: 2 x ~2.0us
  DVE  product p = ex*ey (bf16 TT, 2x mode, ~1.2us), then binary FOLD
       chains (TT add/max over strided 3D halves) + one small reduce:
       sums [128,16,256]->[128,16] ~3.0us, maxes [128,8,256]->[128,8]
       ~1.8us. Total ~6.1us/chunk - the engine pacer at full DMA rate.
First and last chunks run as half-chunks (first also quarter-DMAs) to
shorten pipeline fill/drain. Stats land f32 in [128,384]; tail does
Ln(sums), Ln(maxprod), two row-reduces, subtract -> col 0 of a
[128,1024] padded output (host sums col 0 over cores/partitions).

Hardware facts measured on TRN2 silicon (hold for future edits):
  - DVE TENSOR_TENSOR with ALL-bf16 packed operands runs 2x
    (0.59ns/elem); fp32 or mixed dtype is 1x (1.04). TENSOR_REDUCE is
    ALWAYS 1x regardless of dtype/shape - hence fold-then-small-reduce.
  - GPSIMD software ops (TENSOR_TENSOR) STALL concurrent DVE
    instructions to ~zero throughput for their whole duration
    (1:1 loss) - keep GPSIMD idle, never overlap it with DVE.
  - ACT accum_out costs ~280ns/READ_ACCUMULATOR; [128,1]-granular
    only - useless for per-row sums at T rows/partition.
  - DMAs must issue from nc.sync's ring; ACT's HWDGE ring serializes
    DMAs behind its compute instructions (+15us measured here).
  - A small (512B) SBUF->HBM output transfer sits ~12-14us in the
    write queue (aggregation path) and that lag lands INSIDE the
    profiled exec window; padding the out to 4KB/partition fixes it.
  - vector.tensor_tensor_reduce with op1=max/min WEDGES the core.
  - Custom DVE ops (dve_ops.OPS) all run 1x (no perf_en) and accum_out
    is [128,1]-granular: no win over native TT folds.
"""

import sys

sys.path.insert(0, "/opt/trn_rl_repo")

import numpy as np

import concourse.bacc as bacc
import concourse.mybir as mybir
import concourse.tile as tile
from concourse.bass_utils import run_bass_kernel_spmd

B, C = 131072, 256
N_CORES = 8
R = B // N_CORES          # rows per core = 16384
T = 8                     # rows per partition per chunk
CH = T * C                # 2048 free elems per tensor per chunk
NCH = R // (128 * T)      # 16 chunks per core
NG = R // 128             # 128 row-groups (max values) per core
BUFS = 6                  # input tile ring depth
EBUFS = 3                 # combined-exp tile ring depth (2-chunk supers)
PBUFS = 2                 # product tile ring depth

_cache = {}


def _build_nc(repeat=1):
    f32 = mybir.dt.float32
    bf16 = mybir.dt.bfloat16
    A = mybir.AluOpType
    Exp = mybir.ActivationFunctionType.Exp
    Ln = mybir.ActivationFunctionType.Ln
    X = mybir.AxisListType.X
    nc = bacc.Bacc("TRN2", target_bir_lowering=False, debug=False)
    x_d = nc.dram_tensor("x", [R, C], f32, kind="ExternalInput")
    y_d = nc.dram_tensor("y", [R, C], f32, kind="ExternalInput")
    # [128, 1024] instead of [128, 1]: 512-byte output transfers sit
    # ~12-14us in the SBUF->HBM queue (small-transfer aggregation path);
    # a 4KB-per-partition transfer executes promptly. Host reads col 0.
    out_d = nc.dram_tensor("out", [128, 1024], f32, kind="ExternalOutput")

    # chunk c, partition p holds rows c*1024 + p*T + t (t contiguous)
    x_v = x_d.ap().rearrange("(c p t) f -> c p (t f)", c=NCH, p=128, t=T)
    y_v = y_d.ap().rearrange("(c p t) f -> c p (t f)", c=NCH, p=128, t=T)

    with tile.TileContext(nc) as tc:
        with (
            tc.tile_pool(name="xin", bufs=BUFS) as xin_pool,
            tc.tile_pool(name="yin", bufs=BUFS) as yin_pool,
            tc.tile_pool(name="exp", bufs=EBUFS) as e_pool,
            tc.tile_pool(name="prod", bufs=PBUFS) as p_pool,
            tc.tile_pool(name="fold", bufs=1) as f_pool,
            tc.tile_pool(name="stats", bufs=1) as stats_pool,
        ):
            sxy_t = stats_pool.tile([128, 2 * NG], f32, tag="sxy")
            mx_t = stats_pool.tile([128, NG], f32, tag="mx")
            obig_t = stats_pool.tile([128, 1024], f32, tag="obig")
            nc.vector.memset(obig_t[:], 0.0)


            def half(view_3d, f):
                # [128, t, 2f] -> two [128, t, f] halves
                t = view_3d.shape[1]
                v4 = view_3d.rearrange("p t (h f) -> p t h f", h=2, f=f)
                return v4[:, :, 0], v4[:, :, 1]

            cols = {"sxy": 0, "mx": 0}

            def chain(src_view, nt, cap, op, red, stats_t, ckey, tpre, nfold):
                # src_view: [128, nt*256] 2D AP; nfold bf16 TT folds then
                # one reduce into the next nt cols of stats_t (f32).
                # Fold tiles are fixed cap-size in a bufs=1 pool: safe,
                # all DVE in-order.
                v = src_view.rearrange("p (t f) -> p t f", t=nt)
                f = 128
                for i in range(nfold):
                    t_i = f_pool.tile([128, cap * f], bf16, tag=f"{tpre}{i}")
                    a, b = half(v, f)
                    v = t_i[:, : nt * f].rearrange("p (t f) -> p t f", t=nt)
                    nc.vector.tensor_tensor(out=v, in0=a, in1=b, op=op)
                    f //= 2
                c0 = cols[ckey]
                cols[ckey] = c0 + nt
                red(stats_t[:, c0 : c0 + nt], v, axis=X)

            def sum_chain(e_view, nt):
                chain(e_view, nt, 32, A.add, nc.vector.reduce_sum,
                      sxy_t, "sxy", "s", 3)

            def max_chain(p_view, nt):
                chain(p_view, nt, 16, A.max, nc.vector.reduce_max,
                      mx_t, "mx", "m", 2)

            def half_chunk(c, hh, quarters=False):
                # one half-chunk (4 rows/partition): fast pipeline
                # fill (c==0) and short drain (c==NCH-1). Reuses the
                # full-size tile tags (partial use) to save SBUF.
                hq = CH // 2
                xs = x_v[c][:, hh * hq : (hh + 1) * hq]
                ys = y_v[c][:, hh * hq : (hh + 1) * hq]
                x_t = xin_pool.tile([128, CH], f32, tag="x")
                y_t = yin_pool.tile([128, CH], f32, tag="y")
                if quarters:
                    # interleave x/y quarter-DMAs so both exps can
                    # start as early as possible on the very first chunk
                    q = hq // 2
                    nc.sync.dma_start(x_t[:, :q], xs[:, :q])
                    nc.sync.dma_start(y_t[:, :q], ys[:, :q])
                    nc.sync.dma_start(x_t[:, q:hq], xs[:, q:])
                    nc.sync.dma_start(y_t[:, q:hq], ys[:, q:])
                else:
                    nc.sync.dma_start(x_t[:, :hq], xs)
                    nc.sync.dma_start(y_t[:, :hq], ys)

                e_t = e_pool.tile([128, 4 * CH], bf16, tag="e")
                if quarters:
                    q = hq // 2
                    nc.scalar.activation(e_t[:, :q], x_t[:, :q], Exp)
                    nc.scalar.activation(e_t[:, hq : hq + q], y_t[:, :q], Exp)
                    nc.scalar.activation(e_t[:, q:hq], x_t[:, q:hq], Exp)
                    nc.scalar.activation(e_t[:, hq + q : 2 * hq], y_t[:, q:hq], Exp)
                else:
                    nc.scalar.activation(e_t[:, :hq], x_t[:, :hq], Exp)
                    nc.scalar.activation(e_t[:, hq : 2 * hq], y_t[:, :hq], Exp)

                p_t = p_pool.tile([128, 2 * CH], bf16, tag="p")
                nc.vector.tensor_tensor(
                    out=p_t[:, :hq], in0=e_t[:, :hq],
                    in1=e_t[:, hq : 2 * hq], op=A.mult,
                )
                sum_chain(e_t[:, : 2 * hq], T)
                max_chain(p_t[:, :hq], T // 2)

            def one_pass():
                half_chunk(0, 0, quarters=True)
                half_chunk(0, 1)
                # 2-chunk super-chunks: double-size DVE fold instructions
                # halve the small-instruction overhead and merge the
                # final reduces (~0.5us/chunk off the DVE pacer).
                # Product on DVE: GPSIMD's software TENSOR_TENSOR stalls
                # concurrent DVE instructions to ~zero throughput.
                for s in range(1, NCH - 1, 2):
                    xa = xin_pool.tile([128, CH], f32, tag="x")
                    ya = yin_pool.tile([128, CH], f32, tag="y")
                    nc.sync.dma_start(xa[:], x_v[s])
                    nc.sync.dma_start(ya[:], y_v[s])
                    xb = xin_pool.tile([128, CH], f32, tag="x")
                    yb = yin_pool.tile([128, CH], f32, tag="y")
                    nc.sync.dma_start(xb[:], x_v[s + 1])
                    nc.sync.dma_start(yb[:], y_v[s + 1])

                    # layout [exa | exb | eya | eyb] so the product's
                    # operands are the two contiguous 2-chunk halves
                    e_t = e_pool.tile([128, 4 * CH], bf16, tag="e")
                    nc.scalar.activation(e_t[:, :CH], xa[:], Exp)
                    nc.scalar.activation(e_t[:, 2 * CH : 3 * CH], ya[:], Exp)
                    nc.scalar.activation(e_t[:, CH : 2 * CH], xb[:], Exp)
                    nc.scalar.activation(e_t[:, 3 * CH :], yb[:], Exp)

                    p_t = p_pool.tile([128, 2 * CH], bf16, tag="p")
                    nc.vector.tensor_tensor(
                        out=p_t[:], in0=e_t[:, : 2 * CH],
                        in1=e_t[:, 2 * CH :], op=A.mult,
                    )

                    sum_chain(e_t[:], 4 * T)
                    max_chain(p_t[:], 2 * T)

                # last chunk as two half-chunks to shorten the drain
                half_chunk(NCH - 1, 0)
                # quarter-grain DMAs+exps on the very last half-chunk:
                # lets ACT (and then DVE's final chains) start on the
                # tail data sooner, shortening the drain
                half_chunk(NCH - 1, 1, quarters=True)

            if repeat > 1:
                with tc.For_i(0, repeat, 1):
                    one_pass()
            else:
                one_pass()

            # --- device tail: stats -> [128, 1] partial sum ---
            ln_t = stats_pool.tile([128, 2 * NG], f32, tag="ln")
            mln_t = stats_pool.tile([128, NG], f32, tag="mln")
            ts_t = stats_pool.tile([128, 1], f32, tag="ts")
            tm_t = stats_pool.tile([128, 1], f32, tag="tm")
            nc.scalar.activation(ln_t[:], sxy_t[:], Ln)
            # mx holds max(ex*ey) = exp(max(x+y)); ln() recovers the max
            nc.scalar.activation(mln_t[:], mx_t[:], Ln)
            nc.vector.reduce_sum(ts_t[:], ln_t[:], axis=X)
            nc.vector.reduce_sum(tm_t[:], mln_t[:], axis=X)
            nc.vector.tensor_tensor(
                out=obig_t[:, :1], in0=ts_t[:], in1=tm_t[:], op=A.subtract
            )
            nc.sync.dma_start(out_d.ap(), obig_t[:])

    nc.compile()
    return nc


def get_nc():
    if "nc" not in _cache:
        _cache["nc"] = _build_nc()
    return _cache["nc"]


def run_cores(x, y, **kw):
    nc = get_nc()
    x = np.ascontiguousarray(np.asarray(x, dtype=np.float32))
    y = np.ascontiguousarray(np.asarray(y, dtype=np.float32))
    in_maps = [
        {"x": x[k * R : (k + 1) * R], "y": y[k * R : (k + 1) * R]}
        for k in range(N_CORES)
    ]
    return run_bass_kernel_spmd(nc, in_maps, list(range(N_CORES)), **kw)


def kernel(x, y):
    res = run_cores(x, y)
    total = 0.0
    for r in res.results:
        total += float(np.sum(r["out"][:, 0].astype(np.float64)))
    return np.float32(0.5 * total / B)
